# revision 11
# baseline (speedup 1.0000x reference)
"""Trainium2 Bass kernel for nn_MultiHeadAttention_68659347194437.

Spatial multi-head attention over the W axis (no softmax):
    qp = wq*q + bq ; kp, vp likewise            (1x1 conv over C=32)
    attn = qp @ kp^T  per (b,h)                 [512, 512]
    att  = attn @ vp                            [512, 32]
    out  = att^T + q                            (NCHW residual)

No softmax, so associativity collapses the [512,512] score matrix.
Per head:
    G   = V_aug^T K_aug                  [33,33]  (K_aug = [K; ones])
    A   = [wq|bq]^T [wk|bk]              [33,33]  (constant)
    L   = A (G^T Pv^T) + [I;0]           [33,32]  (Pv = [wv|bv]; +I = residual)
    out = L^T Q_aug                      [32,512]
The L^T Q_aug step dominates PE time if done per head (free dim 512 at
25% array util).  Instead, for each block of 4 heads, the top 32 rows of
L are scattered into a block-diagonal [128,128] stationary L4, and one
matmul computes all 4 heads: out4 = L4^T Q4  (Q4 = 4 heads' channels
stacked on partitions).  The bias row of L (driven by the ones-row of
Q_aug, which no longer fits) is applied as a per-partition scalar add
fused into the PSUM->SBUF output copy.

Sharding: data-parallel over batch B=8 across 8 NeuronCores, no comms.
All input groups get unique SBUF buffers (everything fits), so input
DMAs have no reuse waits and are issued upfront on the three DMA-capable
queues (sync=K, gpsimd=V, scalar=Q); outputs go out per 2 blocks on
sync/scalar alternately.  DMA rows are kept >= 2KB (small-row DMAs pay a
large fixed per-packet cost).  The per-pair compute (2 blocks = 8 heads)
is software-pipelined 4 stages deep so the PE rarely waits on a
PSUM->SBUF copy round trip.
"""

import os
import numpy as np

import concourse.bass as bass
import concourse.bacc as bacc
import concourse.tile as tile
import concourse.mybir as mybir
from concourse.bass_utils import run_bass_kernel_spmd

B, C, H, W = 8, 32, 64, 512
CA = C + 1          # augmented channel dim (ones row/col)
HW = H * W
NCHUNK = HW // 128  # 256 chunks of 128 pixels (4 per head)
NBLK = H // 4       # 16 blocks of 4 heads
NPAIR = NBLK // 2   # 8 pipeline iterations of 2 blocks (8 heads)
GROUPS = [16, 16, 16, 16]  # heads per input-DMA group (4KB+ DMA rows)

BF16 = mybir.dt.bfloat16
F32 = mybir.dt.float32
NP_BF16 = np.dtype(mybir.dt.np(BF16))

# exec time (ns) of the most recent run, when tracing was enabled
last_exec_time_ns = None

_cache = {}


def _build():
    nc = bacc.Bacc(
        "TRN2",
        target_bir_lowering=False,
        debug=False,
        enable_asserts=False,
        num_devices=8,
    )

    q4_d = nc.dram_tensor("q4", [128, NBLK * W], BF16, kind="ExternalInput")
    kt_d = nc.dram_tensor("kta", [128, NCHUNK * CA], BF16, kind="ExternalInput")
    vt_d = nc.dram_tensor("vta", [128, NCHUNK * CA], BF16, kind="ExternalInput")
    # cf: [wq|bq | wk|bk | I33] f32;  cb: [pvt | I32 | I32 tiled x8] bf16
    cf_d = nc.dram_tensor("cf", [CA, 3 * CA], F32, kind="ExternalInput")
    cb_d = nc.dram_tensor("cb", [CA, 2 * C + 256], BF16, kind="ExternalInput")
    out_d = nc.dram_tensor("out", [128, NBLK, W], BF16, kind="ExternalOutput")

    q4 = q4_d.ap()
    kta = kt_d.ap()
    vta = vt_d.ap()
    out_ap = out_d.ap().rearrange("p a w -> p (a w)")

    with tile.TileContext(nc) as tc:
        with (
            tc.tile_pool(name="const", bufs=1) as cpool,
            tc.tile_pool(name="qin", bufs=1) as qpool,
            tc.tile_pool(name="kvin", bufs=1) as kvpool,
            tc.tile_pool(name="l4p", bufs=1) as l4pool,
            tc.tile_pool(name="outp", bufs=1) as opool,
            tc.tile_pool(name="small", bufs=4) as spool,
            tc.tile_pool(name="psg", bufs=2, space=bass.MemorySpace.PSUM) as psg,
            tc.tile_pool(name="psy", bufs=2, space=bass.MemorySpace.PSUM) as psy,
            tc.tile_pool(name="psl", bufs=2, space=bass.MemorySpace.PSUM) as psl,
            tc.tile_pool(name="pso", bufs=2, space=bass.MemorySpace.PSUM) as pso,
        ):
            # ---- constants: one f32 + one bf16 blob, first on the sync queue ----
            cf = cpool.tile([CA, 3 * CA], F32)
            cb = cpool.tile([CA, 2 * C + 256], BF16)
            nc.sync.dma_start(cf[:], cf_d.ap()[:])
            nc.sync.dma_start(cb[:], cb_d.ap()[:])
            wqb = cf[0:C, 0:CA]
            wkb = cf[0:C, CA:2 * CA]
            i33 = cf[:, 2 * CA:3 * CA]
            pvt = cb[:, 0:C]                    # [33, 32] = [wv^T; bv]
            ieye = cb[0:C, C:2 * C]             # [32, 32] identity
            i32t8 = cb[0:C, 2 * C:2 * C + 256]  # [32, 256] = [I x8]

            # ---- input DMAs: all groups upfront, unique buffers ----
            # sync: K
            ktgs, vtgs, qgs = [], [], []
            h0 = 0
            for g, ghn in enumerate(GROUPS):
                ktg = kvpool.tile([128, ghn * 4 * CA], BF16,
                                  tag=f"ktg{g}", name=f"ktg{g}")
                nc.sync.dma_start(ktg[:], kta[:, h0 * 4 * CA:(h0 + ghn) * 4 * CA])
                ktgs.append(ktg)
                h0 += ghn
            # gpsimd: first L4 memset, V, remaining memsets
            l4ps = [
                l4pool.tile([128, 2 * 128], BF16, tag=f"l4_{p}", name=f"l4p{p}")
                for p in range(NPAIR)
            ]
            nc.gpsimd.memset(l4ps[0][:], 0.0)
            h0 = 0
            for g, ghn in enumerate(GROUPS):
                vtg = kvpool.tile([128, ghn * 4 * CA], BF16,
                                  tag=f"vtg{g}", name=f"vtg{g}")
                nc.gpsimd.dma_start(vtg[:], vta[:, h0 * 4 * CA:(h0 + ghn) * 4 * CA])
                vtgs.append(vtg)
                h0 += ghn
            for p in range(1, NPAIR):
                nc.gpsimd.memset(l4ps[p][:], 0.0)
            # scalar: Q (g0/g1 upfront, g2/g3 interleaved into the loop)
            q4_offs = []
            h0 = 0
            for g, ghn in enumerate(GROUPS):
                qg = qpool.tile([128, (ghn // 4) * W], BF16,
                                tag=f"qg{g}", name=f"qg{g}")
                q4_offs.append(((h0 // 4) * W, (h0 // 4 + ghn // 4) * W))
                if g < 2:
                    nc.scalar.dma_start(qg[:], q4[:, q4_offs[g][0]:q4_offs[g][1]])
                qgs.append(qg)
                h0 += ghn

            # ---- A = [wq|bq]^T [wk|bk]; at_sb = A^T (PE transpose via I) ----
            a_ps = psg.tile([CA, CA], F32, tag="g")
            nc.tensor.matmul(a_ps[:], wqb, wkb)
            a_sb = cpool.tile([CA, CA], F32)
            nc.vector.tensor_copy(a_sb[:], a_ps[:])
            at_ps = psg.tile([CA, CA], F32, tag="g")
            nc.tensor.matmul(at_ps[:], a_sb[:], i33)
            at_sb = cpool.tile([CA, CA], BF16)
            nc.vector.tensor_copy(at_sb[:], at_ps[:])

            # ---- software-pipelined main loop over pairs of blocks ----
            # iteration u: G(u), m1(u-1), l~+bias(u-2), out(u-3)
            gt_ps_p = {}
            gt_sb_p = {}
            m1_ps_p = {}
            m1b_p = {}
            lb_ps_p = {}
            biasA_p = {}
            biasB_p = {}
            o_psA_p = {}
            o_psB_p = {}

            for u in range(NPAIR + 3):
                # --- stage A: G for the 8 heads of pair u ---
                if u < NPAIR:
                    g, pu = u // 2, u % 2
                    ktg, vtg = ktgs[g], vtgs[g]
                    gt_ps = psg.tile([CA, 8 * CA], F32, tag="g")
                    for hh in range(8):
                        o0 = ((pu * 8 + hh) * 4) * CA
                        for j in range(4):
                            o = o0 + j * CA
                            nc.tensor.matmul(
                                gt_ps[:, hh * CA:(hh + 1) * CA],
                                vtg[:, o:o + CA],
                                ktg[:, o:o + CA],
                                start=(j == 0),
                                stop=(j == 3),
                            )
                    gt_ps_p[u] = gt_ps

                # --- stage B: m1 = G^T Pv^T (8 heads) for pair u-1 ---
                ub = u - 1
                if 0 <= ub < NPAIR:
                    m1_ps = psy.tile([CA, 8 * C], F32, tag="m1")
                    gt_sb = gt_sb_p[ub]
                    for hh in range(8):
                        nc.tensor.matmul(
                            m1_ps[:, hh * C:(hh + 1) * C],
                            gt_sb[:, hh * CA:(hh + 1) * CA],
                            pvt,
                        )
                    m1_ps_p[ub] = m1_ps

                # --- stage C: l~ = I + (A m1)[:32,:], bias = (A m1)[32,:] ---
                ul = u - 2
                if 0 <= ul < NPAIR:
                    lb_ps = psl.tile([128, 8 * C + 2], F32, tag="l")
                    m1b = m1b_p[ul]
                    nc.tensor.matmul(lb_ps[0:C, 0:8 * C], ieye, i32t8,
                                     start=True, stop=False)
                    nc.tensor.matmul(lb_ps[0:C, 0:8 * C], at_sb[:, 0:C], m1b[:],
                                     start=False, stop=True)
                    nc.tensor.matmul(lb_ps[:, 8 * C:8 * C + 1],
                                     m1b[:, 0:4 * C], at_sb[:, C:C + 1])
                    nc.tensor.matmul(lb_ps[:, 8 * C + 1:8 * C + 2],
                                     m1b[:, 4 * C:8 * C], at_sb[:, C:C + 1])
                    lb_ps_p[ul] = lb_ps

                # --- stage D: out4 = L4^T Q4 for the 2 blocks of pair u-3 ---
                uo = u - 3
                if 0 <= uo < NPAIR:
                    g, pu = uo // 2, uo % 2
                    qg = qgs[g]
                    o_psA = pso.tile([128, W], F32, tag="o")
                    nc.tensor.matmul(o_psA[:], l4ps[uo][:, 0:128],
                                     qg[:, (pu * 2) * W:(pu * 2 + 1) * W])
                    o_psB = pso.tile([128, W], F32, tag="o")
                    nc.tensor.matmul(o_psB[:], l4ps[uo][:, 128:256],
                                     qg[:, (pu * 2 + 1) * W:(pu * 2 + 2) * W])
                    o_psA_p[uo] = o_psA
                    o_psB_p[uo] = o_psB

                # --- copies (emission order fixes per-queue order) ---
                # DVE: diag A (u-2), gt (u), m1b (u-1)
                if 0 <= ul < NPAIR:
                    lb_ps = lb_ps_p[ul]
                    l4p = l4ps[ul]
                    for i in range(4):
                        nc.vector.tensor_copy(
                            l4p[C * i:C * (i + 1), C * i:C * (i + 1)],
                            lb_ps[0:C, C * i:C * (i + 1)])
                if u < NPAIR:
                    gt_sb = spool.tile([CA, 8 * CA], BF16, tag="gt")
                    nc.vector.tensor_copy(gt_sb[:], gt_ps_p[u][:])
                    gt_sb_p[u] = gt_sb
                if 0 <= ub < NPAIR:
                    m1b = spool.tile([CA, 8 * C], BF16, tag="m1b")
                    nc.vector.tensor_copy(m1b[:], m1_ps_p[ub][:])
                    m1b_p[ub] = m1b

                # ACT: q4 prefetch, diag B (u-2), og A (u-3)
                if u < 2:
                    g = u + 2
                    nc.scalar.dma_start(qgs[g][:], q4[:, q4_offs[g][0]:q4_offs[g][1]])
                if 0 <= ul < NPAIR:
                    lb_ps = lb_ps_p[ul]
                    l4p = l4ps[ul]
                    for i in range(4):
                        nc.scalar.copy(
                            l4p[C * i:C * (i + 1), 128 + C * i:128 + C * (i + 1)],
                            lb_ps[0:C, 4 * C + C * i:4 * C + C * (i + 1)])

                # DVE: bias copies (u-2) — gpsimd cannot read PSUM
                if 0 <= ul < NPAIR:
                    lb_ps = lb_ps_p[ul]
                    biasA = spool.tile([128, 1], F32, tag="biasA")
                    biasB = spool.tile([128, 1], F32, tag="biasB")
                    nc.vector.tensor_copy(biasA[:], lb_ps[:, 8 * C:8 * C + 1])
                    nc.vector.tensor_copy(biasB[:], lb_ps[:, 8 * C + 1:8 * C + 2])
                    biasA_p[ul] = biasA
                    biasB_p[ul] = biasB

                # --- output copies (+bias) on ACT, DMA on sync/gpsimd ---
                if 0 <= uo < NPAIR:
                    og = opool.tile([128, 2 * W], BF16, tag=f"og{uo}", name=f"og{uo}")
                    nc.scalar.add(og[:, 0:W], o_psA_p[uo][:], biasA_p[uo][:])
                    nc.scalar.add(og[:, W:2 * W], o_psB_p[uo][:], biasB_p[uo][:])
                    dma_eng = nc.sync if uo % 2 == 0 else nc.gpsimd
                    dma_eng.dma_start(out_ap[:, uo * 2 * W:(uo + 1) * 2 * W], og[:])

    nc.compile()
    return nc


def _prep_core(qb, kb, vb):
    """Host-side relayout for one batch element (one core)."""
    # Q: 4-head channel stacking  [128, NBLK*W], partition = 32*(h%4) + c
    q4 = np.ascontiguousarray(
        qb.reshape(C, NBLK, 4, W).transpose(2, 0, 1, 3)
    ).reshape(128, NBLK * W).astype(NP_BF16)

    def tr(x):
        t = np.empty((HW, CA), dtype=np.float32)
        t[:, :C] = x.reshape(C, HW).T
        t[:, C] = 1.0
        return np.ascontiguousarray(
            t.reshape(NCHUNK, 128, CA).transpose(1, 0, 2)
        ).reshape(128, NCHUNK * CA).astype(NP_BF16)

    return q4, tr(kb), tr(vb)


def _install_ntff_hook():
    """Provide antenv.axon_hooks (absent in this image) so trace=True works."""
    import sys
    import types

    if "antenv.axon_hooks" in sys.modules:
        return
    try:
        import antenv
    except ImportError:
        return
    mod = types.ModuleType("antenv.axon_hooks")
    store = {}
    mod.set_axon_ntff_profile_hook = lambda h: store.__setitem__("h", h)
    mod.get_axon_ntff_profile_hook = lambda: store.get("h")
    sys.modules["antenv.axon_hooks"] = mod
    antenv.axon_hooks = mod
    try:
        from trn_agent_boot.trn_boot import _ntff_profile_via_ctypes

        hook = _ntff_profile_via_ctypes("/opt/axon/libaxon_pjrt.so")
        if hook is not None:
            store["h"] = hook
    except Exception:
        pass


def kernel(q, k, v, wq, bq, wk, bk, wv, bv):
    global last_exec_time_ns
    if "nc" not in _cache:
        _cache["nc"] = _build()
    nc = _cache["nc"]

    q = np.asarray(q, np.float32)
    k = np.asarray(k, np.float32)
    v = np.asarray(v, np.float32)
    wq = np.asarray(wq, np.float32)
    bq = np.asarray(bq, np.float32)
    wk = np.asarray(wk, np.float32)
    bk = np.asarray(bk, np.float32)
    wv = np.asarray(wv, np.float32)
    bv = np.asarray(bv, np.float32)

    cf = np.zeros((CA, 3 * CA), np.float32)
    cf[0:C, 0:CA] = np.concatenate([wq, bq[:, None]], axis=1)
    cf[0:C, CA:2 * CA] = np.concatenate([wk, bk[:, None]], axis=1)
    cf[:, 2 * CA:3 * CA] = np.eye(CA)
    cb = np.zeros((CA, 2 * C + 256), np.float32)
    cb[:, 0:C] = np.concatenate([wv.T, bv[None, :]], axis=0)
    cb[0:C, C:2 * C] = np.eye(C)
    cb[0:C, 2 * C:2 * C + 256] = np.tile(np.eye(C), (1, 8))
    cb = cb.astype(NP_BF16)

    in_maps = []
    for b in range(B):
        q4, kta, vta = _prep_core(q[b], k[b], v[b])
        in_maps.append({
            "q4": q4, "kta": kta, "vta": vta,
            "cf": cf, "cb": cb,
        })

    trace = os.environ.get("KERNEL_TRACE", "0") == "1"
    if trace:
        _install_ntff_hook()
    res = run_bass_kernel_spmd(nc, in_maps, core_ids=list(range(B)), trace=trace)
    last_exec_time_ns = res.exec_time_ns

    outs = []
    for b in range(B):
        arr = np.asarray(res.results[b]["out"], dtype=np.float32).reshape(4, C, H // 4, W)
        outs.append(np.transpose(arr, (1, 2, 0, 3)).reshape(C, H, W))
    return np.stack(outs).astype(np.float32)


# revision 12
# speedup vs baseline: 1.3152x; 1.3152x over previous
"""Trainium2 Bass kernel for nn_MultiHeadAttention_68659347194437.

Spatial multi-head attention over the W axis (no softmax):
    qp = wq*q + bq ; kp, vp likewise            (1x1 conv over C=32)
    attn = qp @ kp^T  per (b,h)                 [512, 512]
    att  = attn @ vp                            [512, 32]
    out  = att^T + q                            (NCHW residual)

No softmax, so associativity collapses the [512,512] score matrix.
Per head:
    G   = V_aug^T K_aug                  [33,33]  (K_aug = [K; ones])
    A   = [wq|bq]^T [wk|bk]              [33,33]  (constant)
    L   = A (G^T Pv^T) + [I;0]           [33,32]  (Pv = [wv|bv]; +I = residual)
    out = L^T Q_aug                      [32,512]
The L^T Q_aug step dominates PE time if done per head (free dim 512 at
25% array util).  Instead, for each block of 4 heads, the top 32 rows of
L are scattered into a block-diagonal [128,128] stationary L4, and one
matmul computes all 4 heads: out4 = L4^T Q4  (Q4 = 4 heads' channels
stacked on partitions).  The bias row of L (driven by the ones-row of
Q_aug, which no longer fits) is applied as a per-partition scalar add
fused into the PSUM->SBUF output copy.

Sharding: data-parallel over batch B=8 across 8 NeuronCores, no comms.
All input groups get unique SBUF buffers (everything fits), so input
DMAs have no reuse waits and are issued upfront on the three DMA-capable
queues (sync=K, gpsimd=V, scalar=Q); outputs go out per 2 blocks on
sync/scalar alternately.  DMA rows are kept >= 2KB (small-row DMAs pay a
large fixed per-packet cost).  The per-pair compute (2 blocks = 8 heads)
is software-pipelined 4 stages deep so the PE rarely waits on a
PSUM->SBUF copy round trip.
"""

import os
import numpy as np

import concourse.bass as bass
import concourse.bacc as bacc
import concourse.tile as tile
import concourse.mybir as mybir
from concourse.bass_utils import run_bass_kernel_spmd

B, C, H, W = 8, 32, 64, 512
CA = C + 1          # augmented channel dim (ones row/col)
HW = H * W
NCHUNK = HW // 128  # 256 chunks of 128 pixels (4 per head)
NBLK = H // 4       # 16 blocks of 4 heads
NPAIR = NBLK // 2   # 8 pipeline iterations of 2 blocks (8 heads)
GROUPS = [16, 16, 16, 16]  # heads per input-DMA group (4KB+ DMA rows)

BF16 = mybir.dt.bfloat16
F32 = mybir.dt.float32
NP_BF16 = np.dtype(mybir.dt.np(BF16))

# exec time (ns) of the most recent run, when tracing was enabled
last_exec_time_ns = None

_cache = {}


def _build():
    nc = bacc.Bacc(
        "TRN2",
        target_bir_lowering=False,
        debug=False,
        enable_asserts=False,
        num_devices=8,
    )

    q4_d = nc.dram_tensor("q4", [128, NBLK * W], BF16, kind="ExternalInput")
    kt_d = nc.dram_tensor("kta", [128, NCHUNK * CA], BF16, kind="ExternalInput")
    vt_d = nc.dram_tensor("vta", [128, NCHUNK * CA], BF16, kind="ExternalInput")
    # cf: [wq|bq | wk|bk | I33] f32;  cb: [pvt | I32 | I32 tiled x8] bf16
    cf_d = nc.dram_tensor("cf", [CA, 3 * CA], F32, kind="ExternalInput")
    cb_d = nc.dram_tensor("cb", [CA, 2 * C + 256], BF16, kind="ExternalInput")
    out_d = nc.dram_tensor("out", [128, NBLK, W], BF16, kind="ExternalOutput")

    q4 = q4_d.ap()
    kta = kt_d.ap()
    vta = vt_d.ap()
    out_ap = out_d.ap().rearrange("p a w -> p (a w)")

    with tile.TileContext(nc) as tc:
        with (
            tc.tile_pool(name="const", bufs=1) as cpool,
            tc.tile_pool(name="qin", bufs=1) as qpool,
            tc.tile_pool(name="kvin", bufs=1) as kvpool,
            tc.tile_pool(name="l4p", bufs=1) as l4pool,
            tc.tile_pool(name="outp", bufs=1) as opool,
            tc.tile_pool(name="small", bufs=4) as spool,
            tc.tile_pool(name="psg", bufs=2, space=bass.MemorySpace.PSUM) as psg,
            tc.tile_pool(name="psy", bufs=2, space=bass.MemorySpace.PSUM) as psy,
            tc.tile_pool(name="psl", bufs=2, space=bass.MemorySpace.PSUM) as psl,
            tc.tile_pool(name="pso", bufs=2, space=bass.MemorySpace.PSUM) as pso,
        ):
            # ---- constants: one f32 + one bf16 blob, first on the scalar
            # queue (small-row DMAs are slow; keep them off the K/V queues) ----
            cf = cpool.tile([CA, 3 * CA], F32)
            cb = cpool.tile([CA, 2 * C + 256], BF16)
            nc.scalar.dma_start(cf[:], cf_d.ap()[:])
            nc.scalar.dma_start(cb[:], cb_d.ap()[:])
            wqb = cf[0:C, 0:CA]
            wkb = cf[0:C, CA:2 * CA]
            i33 = cf[:, 2 * CA:3 * CA]
            pvt = cb[:, 0:C]                    # [33, 32] = [wv^T; bv]
            ieye = cb[0:C, C:2 * C]             # [32, 32] identity
            i32t8 = cb[0:C, 2 * C:2 * C + 256]  # [32, 256] = [I x8]

            # ---- input DMAs: all groups upfront, unique buffers ----
            # sync: K
            ktgs, vtgs, qgs = [], [], []
            h0 = 0
            for g, ghn in enumerate(GROUPS):
                ktg = kvpool.tile([128, ghn * 4 * CA], BF16,
                                  tag=f"ktg{g}", name=f"ktg{g}")
                nc.sync.dma_start(ktg[:], kta[:, h0 * 4 * CA:(h0 + ghn) * 4 * CA])
                ktgs.append(ktg)
                h0 += ghn
            # gpsimd: first L4 memset, V, remaining memsets
            l4ps = [
                l4pool.tile([128, 2 * 128], BF16, tag=f"l4_{p}", name=f"l4p{p}")
                for p in range(NPAIR)
            ]
            nc.gpsimd.memset(l4ps[0][:], 0.0)
            h0 = 0
            for g, ghn in enumerate(GROUPS):
                vtg = kvpool.tile([128, ghn * 4 * CA], BF16,
                                  tag=f"vtg{g}", name=f"vtg{g}")
                nc.gpsimd.dma_start(vtg[:], vta[:, h0 * 4 * CA:(h0 + ghn) * 4 * CA])
                vtgs.append(vtg)
                h0 += ghn
            for p in range(1, NPAIR):
                nc.gpsimd.memset(l4ps[p][:], 0.0)
            # scalar: Q (g0/g1 upfront, g2/g3 interleaved into the loop)
            q4_offs = []
            h0 = 0
            for g, ghn in enumerate(GROUPS):
                qg = qpool.tile([128, (ghn // 4) * W], BF16,
                                tag=f"qg{g}", name=f"qg{g}")
                q4_offs.append(((h0 // 4) * W, (h0 // 4 + ghn // 4) * W))
                if g < 2:
                    nc.scalar.dma_start(qg[:], q4[:, q4_offs[g][0]:q4_offs[g][1]])
                qgs.append(qg)
                h0 += ghn

            # ---- A = [wq|bq]^T [wk|bk]; at_sb = A^T (PE transpose via I) ----
            a_ps = psg.tile([CA, CA], F32, tag="g")
            nc.tensor.matmul(a_ps[:], wqb, wkb)
            a_sb = cpool.tile([CA, CA], F32)
            nc.vector.tensor_copy(a_sb[:], a_ps[:])
            at_ps = psg.tile([CA, CA], F32, tag="g")
            nc.tensor.matmul(at_ps[:], a_sb[:], i33)
            at_sb = cpool.tile([CA, CA], BF16)
            nc.vector.tensor_copy(at_sb[:], at_ps[:])

            # ---- software-pipelined main loop over pairs of blocks ----
            # iteration u: G(u), m1(u-1), l~+bias(u-2), out(u-3)
            gt_ps_p = {}
            gt_sb_p = {}
            m1_ps_p = {}
            m1b_p = {}
            lb_ps_p = {}
            lbsb_p = {}
            biasA_p = {}
            biasB_p = {}
            o_psA_p = {}
            o_psB_p = {}

            for u in range(NPAIR + 3):
                # --- stage A: G for the 8 heads of pair u ---
                if u < NPAIR:
                    g, pu = u // 2, u % 2
                    ktg, vtg = ktgs[g], vtgs[g]
                    gt_ps = psg.tile([CA, 8 * CA], F32, tag="g")
                    for hh in range(8):
                        o0 = ((pu * 8 + hh) * 4) * CA
                        for j in range(4):
                            o = o0 + j * CA
                            nc.tensor.matmul(
                                gt_ps[:, hh * CA:(hh + 1) * CA],
                                vtg[:, o:o + CA],
                                ktg[:, o:o + CA],
                                start=(j == 0),
                                stop=(j == 3),
                            )
                    gt_ps_p[u] = gt_ps

                # --- stage B: m1 = G^T Pv^T (8 heads) for pair u-1 ---
                ub = u - 1
                if 0 <= ub < NPAIR:
                    m1_ps = psy.tile([CA, 8 * C], F32, tag="m1")
                    gt_sb = gt_sb_p[ub]
                    for hh in range(8):
                        nc.tensor.matmul(
                            m1_ps[:, hh * C:(hh + 1) * C],
                            gt_sb[:, hh * CA:(hh + 1) * CA],
                            pvt,
                        )
                    m1_ps_p[ub] = m1_ps

                # --- stage C: l~ = I + (A m1)[:32,:], bias = (A m1)[32,:] ---
                ul = u - 2
                if 0 <= ul < NPAIR:
                    lb_ps = psl.tile([128, 8 * C + 2], F32, tag="l")
                    m1b = m1b_p[ul]
                    nc.tensor.matmul(lb_ps[0:C, 0:8 * C], ieye, i32t8,
                                     start=True, stop=False)
                    nc.tensor.matmul(lb_ps[0:C, 0:8 * C], at_sb[:, 0:C], m1b[:],
                                     start=False, stop=True)
                    nc.tensor.matmul(lb_ps[:, 8 * C:8 * C + 1],
                                     m1b[:, 0:4 * C], at_sb[:, C:C + 1])
                    nc.tensor.matmul(lb_ps[:, 8 * C + 1:8 * C + 2],
                                     m1b[:, 4 * C:8 * C], at_sb[:, C:C + 1])
                    lb_ps_p[ul] = lb_ps

                # --- stage D: out4 = L4^T Q4 for the 2 blocks of pair u-3 ---
                uo = u - 3
                if 0 <= uo < NPAIR:
                    g, pu = uo // 2, uo % 2
                    qg = qgs[g]
                    o_psA = pso.tile([128, W], F32, tag="o")
                    nc.tensor.matmul(o_psA[:], l4ps[uo][:, 0:128],
                                     qg[:, (pu * 2) * W:(pu * 2 + 1) * W])
                    o_psB = pso.tile([128, W], F32, tag="o")
                    nc.tensor.matmul(o_psB[:], l4ps[uo][:, 128:256],
                                     qg[:, (pu * 2 + 1) * W:(pu * 2 + 2) * W])
                    o_psA_p[uo] = o_psA
                    o_psB_p[uo] = o_psB

                # --- copies (emission order fixes per-queue order) ---
                # DVE: l~ -> SBUF (u-2), gt (u), m1b (u-1)
                if 0 <= ul < NPAIR:
                    lbsb = spool.tile([C, 8 * C], BF16, tag="lbsb")
                    nc.vector.tensor_copy(lbsb[:], lb_ps_p[ul][0:C, 0:8 * C])
                    lbsb_p[ul] = lbsb
                if u < NPAIR:
                    gt_sb = spool.tile([CA, 8 * CA], BF16, tag="gt")
                    nc.vector.tensor_copy(gt_sb[:], gt_ps_p[u][:])
                    gt_sb_p[u] = gt_sb
                if 0 <= ub < NPAIR:
                    m1b = spool.tile([CA, 8 * C], BF16, tag="m1b")
                    nc.vector.tensor_copy(m1b[:], m1_ps_p[ub][:])
                    m1b_p[ub] = m1b

                # ACT: q4 prefetch, bias copies (u-2)
                if u < 2:
                    g = u + 2
                    nc.scalar.dma_start(qgs[g][:], q4[:, q4_offs[g][0]:q4_offs[g][1]])
                if 0 <= ul < NPAIR:
                    lb_ps = lb_ps_p[ul]
                    biasA = spool.tile([128, 1], F32, tag="biasA")
                    biasB = spool.tile([128, 1], F32, tag="biasB")
                    nc.scalar.copy(biasA[:], lb_ps[:, 8 * C:8 * C + 1])
                    nc.scalar.copy(biasB[:], lb_ps[:, 8 * C + 1:8 * C + 2])
                    biasA_p[ul] = biasA
                    biasB_p[ul] = biasB

                # GpSimd: SBUF->SBUF block-diag scatter of l~ into L4 (u-2)
                if 0 <= ul < NPAIR:
                    lbsb3 = lbsb_p[ul].rearrange("p (b c) -> p b c", b=2)
                    l4p3 = l4ps[ul].rearrange("p (b c) -> p b c", b=2)
                    for i in range(4):
                        nc.gpsimd.tensor_copy(
                            l4p3[C * i:C * (i + 1), :, C * i:C * (i + 1)],
                            lbsb3[0:C, :, C * i:C * (i + 1)])

                # --- output copies (+bias): og A on DVE, og B on ACT ---
                if 0 <= uo < NPAIR:
                    og = opool.tile([128, 2 * W], BF16, tag=f"og{uo}", name=f"og{uo}")
                    nc.vector.tensor_scalar_add(og[:, 0:W], o_psA_p[uo][:],
                                                biasA_p[uo][:])
                    nc.scalar.add(og[:, W:2 * W], o_psB_p[uo][:], biasB_p[uo][:])
                    nc.sync.dma_start(out_ap[:, uo * 2 * W:(uo + 1) * 2 * W], og[:])

    nc.compile()
    return nc


def _prep_core(qb, kb, vb):
    """Host-side relayout for one batch element (one core)."""
    # Q: 4-head channel stacking  [128, NBLK*W], partition = 32*(h%4) + c
    q4 = np.ascontiguousarray(
        qb.reshape(C, NBLK, 4, W).transpose(2, 0, 1, 3)
    ).reshape(128, NBLK * W).astype(NP_BF16)

    def tr(x):
        t = np.empty((HW, CA), dtype=np.float32)
        t[:, :C] = x.reshape(C, HW).T
        t[:, C] = 1.0
        return np.ascontiguousarray(
            t.reshape(NCHUNK, 128, CA).transpose(1, 0, 2)
        ).reshape(128, NCHUNK * CA).astype(NP_BF16)

    return q4, tr(kb), tr(vb)


def _install_ntff_hook():
    """Provide antenv.axon_hooks (absent in this image) so trace=True works."""
    import sys
    import types

    if "antenv.axon_hooks" in sys.modules:
        return
    try:
        import antenv
    except ImportError:
        return
    mod = types.ModuleType("antenv.axon_hooks")
    store = {}
    mod.set_axon_ntff_profile_hook = lambda h: store.__setitem__("h", h)
    mod.get_axon_ntff_profile_hook = lambda: store.get("h")
    sys.modules["antenv.axon_hooks"] = mod
    antenv.axon_hooks = mod
    try:
        from trn_agent_boot.trn_boot import _ntff_profile_via_ctypes

        hook = _ntff_profile_via_ctypes("/opt/axon/libaxon_pjrt.so")
        if hook is not None:
            store["h"] = hook
    except Exception:
        pass


def kernel(q, k, v, wq, bq, wk, bk, wv, bv):
    global last_exec_time_ns
    if "nc" not in _cache:
        _cache["nc"] = _build()
    nc = _cache["nc"]

    q = np.asarray(q, np.float32)
    k = np.asarray(k, np.float32)
    v = np.asarray(v, np.float32)
    wq = np.asarray(wq, np.float32)
    bq = np.asarray(bq, np.float32)
    wk = np.asarray(wk, np.float32)
    bk = np.asarray(bk, np.float32)
    wv = np.asarray(wv, np.float32)
    bv = np.asarray(bv, np.float32)

    cf = np.zeros((CA, 3 * CA), np.float32)
    cf[0:C, 0:CA] = np.concatenate([wq, bq[:, None]], axis=1)
    cf[0:C, CA:2 * CA] = np.concatenate([wk, bk[:, None]], axis=1)
    cf[:, 2 * CA:3 * CA] = np.eye(CA)
    cb = np.zeros((CA, 2 * C + 256), np.float32)
    cb[:, 0:C] = np.concatenate([wv.T, bv[None, :]], axis=0)
    cb[0:C, C:2 * C] = np.eye(C)
    cb[0:C, 2 * C:2 * C + 256] = np.tile(np.eye(C), (1, 8))
    cb = cb.astype(NP_BF16)

    in_maps = []
    for b in range(B):
        q4, kta, vta = _prep_core(q[b], k[b], v[b])
        in_maps.append({
            "q4": q4, "kta": kta, "vta": vta,
            "cf": cf, "cb": cb,
        })

    trace = os.environ.get("KERNEL_TRACE", "0") == "1"
    if trace:
        _install_ntff_hook()
    res = run_bass_kernel_spmd(nc, in_maps, core_ids=list(range(B)), trace=trace)
    last_exec_time_ns = res.exec_time_ns

    outs = []
    for b in range(B):
        arr = np.asarray(res.results[b]["out"], dtype=np.float32).reshape(4, C, H // 4, W)
        outs.append(np.transpose(arr, (1, 2, 0, 3)).reshape(C, H, W))
    return np.stack(outs).astype(np.float32)


# revision 13
# speedup vs baseline: 1.4385x; 1.0937x over previous
"""Trainium2 Bass kernel for nn_MultiHeadAttention_68659347194437.

Spatial multi-head attention over the W axis (no softmax):
    qp = wq*q + bq ; kp, vp likewise            (1x1 conv over C=32)
    attn = qp @ kp^T  per (b,h)                 [512, 512]
    att  = attn @ vp                            [512, 32]
    out  = att^T + q                            (NCHW residual)

No softmax, so associativity collapses the [512,512] score matrix.
Per head:
    G   = V_aug^T K_aug                  [33,33]  (K_aug = [K; ones])
    A   = [wq|bq]^T [wk|bk]              [33,33]  (constant)
    L   = A (G^T Pv^T) + [I;0]           [33,32]  (Pv = [wv|bv]; +I = residual)
    out = L^T Q_aug                      [32,512]
The L^T Q_aug step dominates PE time if done per head (free dim 512 at
25% array util).  Instead, for each block of 4 heads, the top 32 rows of
L are scattered into a block-diagonal [128,128] stationary L4, and one
matmul computes all 4 heads: out4 = L4^T Q4  (Q4 = 4 heads' channels
stacked on partitions).  The bias row of L (driven by the ones-row of
Q_aug, which no longer fits) is applied as a per-partition scalar add
fused into the PSUM->SBUF output copy.

Sharding: data-parallel over batch B=8 across 8 NeuronCores, no comms.
All input groups get unique SBUF buffers (everything fits), so input
DMAs have no reuse waits and are issued upfront on the three DMA-capable
queues (sync=K, gpsimd=V, scalar=Q); outputs go out per 2 blocks on
sync/scalar alternately.  DMA rows are kept >= 2KB (small-row DMAs pay a
large fixed per-packet cost).  The per-pair compute (2 blocks = 8 heads)
is software-pipelined 4 stages deep so the PE rarely waits on a
PSUM->SBUF copy round trip.
"""

import os
import numpy as np

import concourse.bass as bass
import concourse.bacc as bacc
import concourse.tile as tile
import concourse.mybir as mybir
from concourse.bass_utils import run_bass_kernel_spmd

B, C, H, W = 8, 32, 64, 512
CA = C + 1          # augmented channel dim (ones row/col)
HW = H * W
NCHUNK = HW // 128  # 256 chunks of 128 pixels (4 per head)
NBLK = H // 4       # 16 blocks of 4 heads
NPAIR = NBLK // 2   # 8 pipeline iterations of 2 blocks (8 heads)
GROUPS = [16, 16, 16, 16]  # heads per input-DMA group (4KB+ DMA rows)

BF16 = mybir.dt.bfloat16
F32 = mybir.dt.float32
NP_BF16 = np.dtype(mybir.dt.np(BF16))

# exec time (ns) of the most recent run, when tracing was enabled
last_exec_time_ns = None

_cache = {}


def _build():
    nc = bacc.Bacc(
        "TRN2",
        target_bir_lowering=False,
        debug=False,
        enable_asserts=False,
        num_devices=8,
    )

    q4_d = nc.dram_tensor("q4", [128, NBLK * W], BF16, kind="ExternalInput")
    kt_d = nc.dram_tensor("kta", [128, NCHUNK * CA], BF16, kind="ExternalInput")
    vt_d = nc.dram_tensor("vta", [128, NCHUNK * CA], BF16, kind="ExternalInput")
    # cf: [wq|bq | wk|bk | I33] f32;  cb: [pvt | I32 | I32 tiled x8] bf16
    cf_d = nc.dram_tensor("cf", [CA, 3 * CA], F32, kind="ExternalInput")
    cb_d = nc.dram_tensor("cb", [CA, 2 * C + 256], BF16, kind="ExternalInput")
    out_d = nc.dram_tensor("out", [128, NBLK, W], BF16, kind="ExternalOutput")

    q4 = q4_d.ap()
    kta = kt_d.ap()
    vta = vt_d.ap()
    out_ap = out_d.ap().rearrange("p a w -> p (a w)")

    with tile.TileContext(nc) as tc:
        with (
            tc.tile_pool(name="const", bufs=1) as cpool,
            tc.tile_pool(name="qin", bufs=1) as qpool,
            tc.tile_pool(name="kvin", bufs=1) as kvpool,
            tc.tile_pool(name="l4p", bufs=1) as l4pool,
            tc.tile_pool(name="outp", bufs=1) as opool,
            tc.tile_pool(name="small", bufs=4) as spool,
            tc.tile_pool(name="psg", bufs=2, space=bass.MemorySpace.PSUM) as psg,
            tc.tile_pool(name="psy", bufs=2, space=bass.MemorySpace.PSUM) as psy,
            tc.tile_pool(name="psl", bufs=2, space=bass.MemorySpace.PSUM) as psl,
            tc.tile_pool(name="pso", bufs=2, space=bass.MemorySpace.PSUM) as pso,
        ):
            # ---- constants: one f32 + one bf16 blob, first on the scalar
            # queue (small-row DMAs are slow; keep them off the K/V queues) ----
            cf = cpool.tile([CA, 3 * CA], F32)
            cb = cpool.tile([CA, 2 * C + 256], BF16)
            nc.scalar.dma_start(cf[:], cf_d.ap()[:])
            nc.scalar.dma_start(cb[:], cb_d.ap()[:])
            wqb = cf[0:C, 0:CA]
            wkb = cf[0:C, CA:2 * CA]
            i33 = cf[:, 2 * CA:3 * CA]
            pvt = cb[:, 0:C]                    # [33, 32] = [wv^T; bv]
            ieye = cb[0:C, C:2 * C]             # [32, 32] identity
            i32t8 = cb[0:C, 2 * C:2 * C + 256]  # [32, 256] = [I x8]

            # ---- input DMAs: all groups upfront, unique buffers ----
            # sync: K
            ktgs, vtgs, qgs = [], [], []
            h0 = 0
            for g, ghn in enumerate(GROUPS):
                ktg = kvpool.tile([128, ghn * 4 * CA], BF16,
                                  tag=f"ktg{g}", name=f"ktg{g}")
                nc.sync.dma_start(ktg[:], kta[:, h0 * 4 * CA:(h0 + ghn) * 4 * CA])
                ktgs.append(ktg)
                h0 += ghn
            # gpsimd: first L4 memset, V, remaining memsets
            l4ps = [
                l4pool.tile([128, 2 * 128], BF16, tag=f"l4_{p}", name=f"l4p{p}")
                for p in range(NPAIR)
            ]
            h0 = 0
            for g, ghn in enumerate(GROUPS):
                vtg = kvpool.tile([128, ghn * 4 * CA], BF16,
                                  tag=f"vtg{g}", name=f"vtg{g}")
                nc.gpsimd.dma_start(vtg[:], vta[:, h0 * 4 * CA:(h0 + ghn) * 4 * CA])
                vtgs.append(vtg)
                h0 += ghn
            for p in range(NPAIR):
                nc.gpsimd.memset(l4ps[p][:], 0.0)
            # scalar: Q (g0/g1 upfront, g2/g3 interleaved into the loop)
            q4_offs = []
            h0 = 0
            for g, ghn in enumerate(GROUPS):
                qg = qpool.tile([128, (ghn // 4) * W], BF16,
                                tag=f"qg{g}", name=f"qg{g}")
                q4_offs.append(((h0 // 4) * W, (h0 // 4 + ghn // 4) * W))
                nc.scalar.dma_start(qg[:], q4[:, q4_offs[g][0]:q4_offs[g][1]])
                qgs.append(qg)
                h0 += ghn

            # ---- A = [wq|bq]^T [wk|bk]; at_sb = A^T (PE transpose via I) ----
            a_ps = psg.tile([CA, CA], F32, tag="g")
            nc.tensor.matmul(a_ps[:], wqb, wkb)
            a_sb = cpool.tile([CA, CA], F32)
            nc.vector.tensor_copy(a_sb[:], a_ps[:])
            at_ps = psg.tile([CA, CA], F32, tag="g")
            nc.tensor.matmul(at_ps[:], a_sb[:], i33)
            at_sb = cpool.tile([CA, CA], BF16)
            nc.vector.tensor_copy(at_sb[:], at_ps[:])

            # ---- software-pipelined main loop over pairs of blocks ----
            # iteration u: G(u), m1(u-1), l~+bias(u-2), out(u-3)
            gt_ps_p = {}
            gt_sb_p = {}
            m1_ps_p = {}
            m1b_p = {}
            lb_ps_p = {}
            lbsb_p = {}
            biasA_p = {}
            biasB_p = {}
            o_psA_p = {}
            o_psB_p = {}

            for u in range(NPAIR + 4):
                # --- stage A: G for the 8 heads of pair u ---
                if u < NPAIR:
                    g, pu = u // 2, u % 2
                    ktg, vtg = ktgs[g], vtgs[g]
                    gt_ps = psg.tile([CA, 8 * CA], F32, tag="g")
                    for hh in range(8):
                        o0 = ((pu * 8 + hh) * 4) * CA
                        for j in range(4):
                            o = o0 + j * CA
                            nc.tensor.matmul(
                                gt_ps[:, hh * CA:(hh + 1) * CA],
                                vtg[:, o:o + CA],
                                ktg[:, o:o + CA],
                                start=(j == 0),
                                stop=(j == 3),
                            )
                    gt_ps_p[u] = gt_ps

                # --- stage B: m1 = G^T Pv^T (8 heads) for pair u-1 ---
                ub = u - 1
                if 0 <= ub < NPAIR:
                    m1_ps = psy.tile([CA, 8 * C], F32, tag="m1")
                    gt_sb = gt_sb_p[ub]
                    for hh in range(8):
                        nc.tensor.matmul(
                            m1_ps[:, hh * C:(hh + 1) * C],
                            gt_sb[:, hh * CA:(hh + 1) * CA],
                            pvt,
                        )
                    m1_ps_p[ub] = m1_ps

                # --- stage C: l~ = I + (A m1)[:32,:], bias = (A m1)[32,:] ---
                ul = u - 2
                if 0 <= ul < NPAIR:
                    lb_ps = psl.tile([128, 8 * C + 2], F32, tag="l")
                    m1b = m1b_p[ul]
                    nc.tensor.matmul(lb_ps[0:C, 0:8 * C], ieye, i32t8,
                                     start=True, stop=False)
                    nc.tensor.matmul(lb_ps[0:C, 0:8 * C], at_sb[:, 0:C], m1b[:],
                                     start=False, stop=True)
                    nc.tensor.matmul(lb_ps[:, 8 * C:8 * C + 1],
                                     m1b[:, 0:4 * C], at_sb[:, C:C + 1])
                    nc.tensor.matmul(lb_ps[:, 8 * C + 1:8 * C + 2],
                                     m1b[:, 4 * C:8 * C], at_sb[:, C:C + 1])
                    lb_ps_p[ul] = lb_ps

                # --- stage D: out4 = L4^T Q4 for the 2 blocks of pair u-4 ---
                uo = u - 4
                if 0 <= uo < NPAIR:
                    g, pu = uo // 2, uo % 2
                    qg = qgs[g]
                    o_psA = pso.tile([128, W], F32, tag="o")
                    nc.tensor.matmul(o_psA[:], l4ps[uo][:, 0:128],
                                     qg[:, (pu * 2) * W:(pu * 2 + 1) * W])
                    o_psB = pso.tile([128, W], F32, tag="o")
                    nc.tensor.matmul(o_psB[:], l4ps[uo][:, 128:256],
                                     qg[:, (pu * 2 + 1) * W:(pu * 2 + 2) * W])
                    o_psA_p[uo] = o_psA
                    o_psB_p[uo] = o_psB

                # --- copies, DVE in PE-completion order:
                #     gt(u), m1b(u-1), lbsb(u-2), scatter(u-2), bias(u-2)
                if u < NPAIR:
                    gt_sb = spool.tile([CA, 8 * CA], BF16, tag="gt")
                    nc.vector.tensor_copy(gt_sb[:], gt_ps_p[u][:])
                    gt_sb_p[u] = gt_sb
                if 0 <= ub < NPAIR:
                    m1b = spool.tile([CA, 8 * C], BF16, tag="m1b")
                    nc.vector.tensor_copy(m1b[:], m1_ps_p[ub][:])
                    m1b_p[ub] = m1b
                if 0 <= ul < NPAIR:
                    lb_ps = lb_ps_p[ul]
                    lbsb = spool.tile([C, 8 * C], BF16, tag="lbsb")
                    nc.vector.tensor_copy(lbsb[:], lb_ps[0:C, 0:8 * C])
                    lbsb3 = lbsb.rearrange("p (b c) -> p b c", b=2)
                    l4p3 = l4ps[ul].rearrange("p (b c) -> p b c", b=2)
                    for i in range(4):
                        nc.vector.tensor_copy(
                            l4p3[C * i:C * (i + 1), :, C * i:C * (i + 1)],
                            lbsb3[0:C, :, C * i:C * (i + 1)])
                    biasA = spool.tile([128, 1], F32, tag="biasA")
                    biasB = spool.tile([128, 1], F32, tag="biasB")
                    nc.vector.tensor_copy(biasA[:], lb_ps[:, 8 * C:8 * C + 1])
                    nc.vector.tensor_copy(biasB[:], lb_ps[:, 8 * C + 1:8 * C + 2])
                    biasA_p[ul] = biasA
                    biasB_p[ul] = biasB

                # --- output copies (+bias) on ACT, DMA on sync ---
                if 0 <= uo < NPAIR:
                    og = opool.tile([128, 2 * W], BF16, tag=f"og{uo}", name=f"og{uo}")
                    nc.scalar.add(og[:, 0:W], o_psA_p[uo][:], biasA_p[uo][:])
                    nc.scalar.add(og[:, W:2 * W], o_psB_p[uo][:], biasB_p[uo][:])
                    nc.sync.dma_start(out_ap[:, uo * 2 * W:(uo + 1) * 2 * W], og[:])

    nc.compile()
    return nc


def _prep_core(qb, kb, vb):
    """Host-side relayout for one batch element (one core)."""
    # Q: 4-head channel stacking  [128, NBLK*W], partition = 32*(h%4) + c
    q4 = np.ascontiguousarray(
        qb.reshape(C, NBLK, 4, W).transpose(2, 0, 1, 3)
    ).reshape(128, NBLK * W).astype(NP_BF16)

    def tr(x):
        t = np.empty((HW, CA), dtype=np.float32)
        t[:, :C] = x.reshape(C, HW).T
        t[:, C] = 1.0
        return np.ascontiguousarray(
            t.reshape(NCHUNK, 128, CA).transpose(1, 0, 2)
        ).reshape(128, NCHUNK * CA).astype(NP_BF16)

    return q4, tr(kb), tr(vb)


def _install_ntff_hook():
    """Provide antenv.axon_hooks (absent in this image) so trace=True works."""
    import sys
    import types

    if "antenv.axon_hooks" in sys.modules:
        return
    try:
        import antenv
    except ImportError:
        return
    mod = types.ModuleType("antenv.axon_hooks")
    store = {}
    mod.set_axon_ntff_profile_hook = lambda h: store.__setitem__("h", h)
    mod.get_axon_ntff_profile_hook = lambda: store.get("h")
    sys.modules["antenv.axon_hooks"] = mod
    antenv.axon_hooks = mod
    try:
        from trn_agent_boot.trn_boot import _ntff_profile_via_ctypes

        hook = _ntff_profile_via_ctypes("/opt/axon/libaxon_pjrt.so")
        if hook is not None:
            store["h"] = hook
    except Exception:
        pass


def kernel(q, k, v, wq, bq, wk, bk, wv, bv):
    global last_exec_time_ns
    if "nc" not in _cache:
        _cache["nc"] = _build()
    nc = _cache["nc"]

    q = np.asarray(q, np.float32)
    k = np.asarray(k, np.float32)
    v = np.asarray(v, np.float32)
    wq = np.asarray(wq, np.float32)
    bq = np.asarray(bq, np.float32)
    wk = np.asarray(wk, np.float32)
    bk = np.asarray(bk, np.float32)
    wv = np.asarray(wv, np.float32)
    bv = np.asarray(bv, np.float32)

    cf = np.zeros((CA, 3 * CA), np.float32)
    cf[0:C, 0:CA] = np.concatenate([wq, bq[:, None]], axis=1)
    cf[0:C, CA:2 * CA] = np.concatenate([wk, bk[:, None]], axis=1)
    cf[:, 2 * CA:3 * CA] = np.eye(CA)
    cb = np.zeros((CA, 2 * C + 256), np.float32)
    cb[:, 0:C] = np.concatenate([wv.T, bv[None, :]], axis=0)
    cb[0:C, C:2 * C] = np.eye(C)
    cb[0:C, 2 * C:2 * C + 256] = np.tile(np.eye(C), (1, 8))
    cb = cb.astype(NP_BF16)

    in_maps = []
    for b in range(B):
        q4, kta, vta = _prep_core(q[b], k[b], v[b])
        in_maps.append({
            "q4": q4, "kta": kta, "vta": vta,
            "cf": cf, "cb": cb,
        })

    trace = os.environ.get("KERNEL_TRACE", "0") == "1"
    if trace:
        _install_ntff_hook()
    res = run_bass_kernel_spmd(nc, in_maps, core_ids=list(range(B)), trace=trace)
    last_exec_time_ns = res.exec_time_ns

    outs = []
    for b in range(B):
        arr = np.asarray(res.results[b]["out"], dtype=np.float32).reshape(4, C, H // 4, W)
        outs.append(np.transpose(arr, (1, 2, 0, 3)).reshape(C, H, W))
    return np.stack(outs).astype(np.float32)


# revision 14
# speedup vs baseline: 1.4404x; 1.0013x over previous
"""Trainium2 Bass kernel for nn_MultiHeadAttention_68659347194437.

Spatial multi-head attention over the W axis (no softmax):
    qp = wq*q + bq ; kp, vp likewise            (1x1 conv over C=32)
    attn = qp @ kp^T  per (b,h)                 [512, 512]
    att  = attn @ vp                            [512, 32]
    out  = att^T + q                            (NCHW residual)

No softmax, so associativity collapses the [512,512] score matrix.
Per head:
    G   = V_aug^T K_aug                  [33,33]  (K_aug = [K; ones])
    A   = [wq|bq]^T [wk|bk]              [33,33]  (constant)
    L   = A (G^T Pv^T) + [I;0]           [33,32]  (Pv = [wv|bv]; +I = residual)
    out = L^T Q_aug                      [32,512]
The L^T Q_aug step dominates PE time if done per head (free dim 512 at
25% array util).  Instead, for each block of 4 heads, the top 32 rows of
L are scattered into a block-diagonal [128,128] stationary L4, and one
matmul computes all 4 heads: out4 = L4^T Q4  (Q4 = 4 heads' channels
stacked on partitions).  The bias row of L (driven by the ones-row of
Q_aug, which no longer fits) is applied as a per-partition scalar add
fused into the PSUM->SBUF output copy.

Sharding: data-parallel over batch B=8 across 8 NeuronCores, no comms.
All input groups get unique SBUF buffers (everything fits), so input
DMAs have no reuse waits and are issued upfront on the three DMA-capable
queues (sync=K, gpsimd=V, scalar=Q); outputs go out per 2 blocks on
sync/scalar alternately.  DMA rows are kept >= 2KB (small-row DMAs pay a
large fixed per-packet cost).  The per-pair compute (2 blocks = 8 heads)
is software-pipelined 4 stages deep so the PE rarely waits on a
PSUM->SBUF copy round trip.
"""

import os
import numpy as np

import concourse.bass as bass
import concourse.bacc as bacc
import concourse.tile as tile
import concourse.mybir as mybir
from concourse.bass_utils import run_bass_kernel_spmd

B, C, H, W = 8, 32, 64, 512
CA = C + 1          # augmented channel dim (ones row/col)
HW = H * W
NCHUNK = HW // 128  # 256 chunks of 128 pixels (4 per head)
NBLK = H // 4       # 16 blocks of 4 heads
NPAIR = NBLK // 2   # 8 pipeline iterations of 2 blocks (8 heads)
GROUPS = [16, 16, 16, 16]  # heads per input-DMA group (4KB+ DMA rows)

BF16 = mybir.dt.bfloat16
F32 = mybir.dt.float32
NP_BF16 = np.dtype(mybir.dt.np(BF16))

# exec time (ns) of the most recent run, when tracing was enabled
last_exec_time_ns = None

_cache = {}


def _build():
    nc = bacc.Bacc(
        "TRN2",
        target_bir_lowering=False,
        debug=False,
        enable_asserts=False,
        num_devices=8,
    )

    q4_d = nc.dram_tensor("q4", [128, NBLK * W], BF16, kind="ExternalInput")
    kt_d = nc.dram_tensor("kta", [128, NCHUNK * CA], BF16, kind="ExternalInput")
    vt_d = nc.dram_tensor("vta", [128, NCHUNK * CA], BF16, kind="ExternalInput")
    # cf: [wq|bq | wk|bk | I33] f32;  cb: [pvt | I32 | I32 tiled x8] bf16
    cf_d = nc.dram_tensor("cf", [CA, 3 * CA], F32, kind="ExternalInput")
    cb_d = nc.dram_tensor("cb", [CA, 2 * C + 256], BF16, kind="ExternalInput")
    out_d = nc.dram_tensor("out", [128, NBLK, W], BF16, kind="ExternalOutput")

    q4 = q4_d.ap()
    kta = kt_d.ap()
    vta = vt_d.ap()
    out_ap = out_d.ap().rearrange("p a w -> p (a w)")

    with tile.TileContext(nc) as tc:
        with (
            tc.tile_pool(name="const", bufs=1) as cpool,
            tc.tile_pool(name="qin", bufs=1) as qpool,
            tc.tile_pool(name="kvin", bufs=1) as kvpool,
            tc.tile_pool(name="l4p", bufs=1) as l4pool,
            tc.tile_pool(name="outp", bufs=1) as opool,
            tc.tile_pool(name="small", bufs=4) as spool,
            tc.tile_pool(name="psg", bufs=2, space=bass.MemorySpace.PSUM) as psg,
            tc.tile_pool(name="psy", bufs=2, space=bass.MemorySpace.PSUM) as psy,
            tc.tile_pool(name="psl", bufs=2, space=bass.MemorySpace.PSUM) as psl,
            tc.tile_pool(name="pso", bufs=2, space=bass.MemorySpace.PSUM) as pso,
        ):
            # ---- constants: one f32 + one bf16 blob, first on the scalar
            # queue (small-row DMAs are slow; keep them off the K/V queues) ----
            cf = cpool.tile([CA, 3 * CA], F32)
            cb = cpool.tile([CA, 2 * C + 256], BF16)
            nc.scalar.dma_start(cf[:], cf_d.ap()[:])
            nc.scalar.dma_start(cb[:], cb_d.ap()[:])
            wqb = cf[0:C, 0:CA]
            wkb = cf[0:C, CA:2 * CA]
            i33 = cf[:, 2 * CA:3 * CA]
            pvt = cb[:, 0:C]                    # [33, 32] = [wv^T; bv]
            ieye = cb[0:C, C:2 * C]             # [32, 32] identity
            i32t8 = cb[0:C, 2 * C:2 * C + 256]  # [32, 256] = [I x8]

            # ---- input DMAs: all groups upfront, unique buffers ----
            # sync: K
            ktgs, vtgs, qgs = [], [], []
            h0 = 0
            for g, ghn in enumerate(GROUPS):
                ktg = kvpool.tile([128, ghn * 4 * CA], BF16,
                                  tag=f"ktg{g}", name=f"ktg{g}")
                nc.sync.dma_start(ktg[:], kta[:, h0 * 4 * CA:(h0 + ghn) * 4 * CA])
                ktgs.append(ktg)
                h0 += ghn
            # gpsimd: first L4 memset, V, remaining memsets
            l4ps = [
                l4pool.tile([128, 2 * 128], BF16, tag=f"l4_{p}", name=f"l4p{p}")
                for p in range(NPAIR)
            ]
            h0 = 0
            for g, ghn in enumerate(GROUPS):
                vtg = kvpool.tile([128, ghn * 4 * CA], BF16,
                                  tag=f"vtg{g}", name=f"vtg{g}")
                nc.gpsimd.dma_start(vtg[:], vta[:, h0 * 4 * CA:(h0 + ghn) * 4 * CA])
                vtgs.append(vtg)
                h0 += ghn
            for p in range(NPAIR):
                nc.gpsimd.memset(l4ps[p][:], 0.0)
            # scalar: Q (g0/g1 upfront, g2/g3 interleaved into the loop)
            q4_offs = []
            h0 = 0
            for g, ghn in enumerate(GROUPS):
                qg = qpool.tile([128, (ghn // 4) * W], BF16,
                                tag=f"qg{g}", name=f"qg{g}")
                q4_offs.append(((h0 // 4) * W, (h0 // 4 + ghn // 4) * W))
                nc.scalar.dma_start(qg[:], q4[:, q4_offs[g][0]:q4_offs[g][1]])
                qgs.append(qg)
                h0 += ghn

            # ---- A = [wq|bq]^T [wk|bk]; at_sb = A^T (PE transpose via I) ----
            a_ps = psg.tile([CA, CA], F32, tag="g")
            nc.tensor.matmul(a_ps[:], wqb, wkb)
            a_sb = cpool.tile([CA, CA], F32)
            nc.vector.tensor_copy(a_sb[:], a_ps[:])
            at_ps = psg.tile([CA, CA], F32, tag="g")
            nc.tensor.matmul(at_ps[:], a_sb[:], i33)
            at_sb = cpool.tile([CA, CA], BF16)
            nc.vector.tensor_copy(at_sb[:], at_ps[:])

            # ---- software-pipelined main loop over pairs of blocks ----
            # iteration u: G(u), m1(u-1), l~+bias(u-2), out(u-3)
            gt_ps_p = {}
            gt_sb_p = {}
            m1_ps_p = {}
            m1b_p = {}
            lb_ps_p = {}
            lbsb_p = {}
            biasA_p = {}
            biasB_p = {}
            o_psA_p = {}
            o_psB_p = {}

            for u in range(NPAIR + 4):
                # --- stage A: G for the 8 heads of pair u ---
                if u < NPAIR:
                    g, pu = u // 2, u % 2
                    ktg, vtg = ktgs[g], vtgs[g]
                    gt_ps = psg.tile([CA, 8 * CA], F32, tag="g")
                    for hh in range(8):
                        o0 = ((pu * 8 + hh) * 4) * CA
                        for j in range(4):
                            o = o0 + j * CA
                            nc.tensor.matmul(
                                gt_ps[:, hh * CA:(hh + 1) * CA],
                                vtg[:, o:o + CA],
                                ktg[:, o:o + CA],
                                start=(j == 0),
                                stop=(j == 3),
                            )
                    gt_ps_p[u] = gt_ps

                # --- stage B: m1 = G^T Pv^T (8 heads) for pair u-1 ---
                ub = u - 1
                if 0 <= ub < NPAIR:
                    m1_ps = psy.tile([CA, 8 * C], F32, tag="m1")
                    gt_sb = gt_sb_p[ub]
                    for hh in range(8):
                        nc.tensor.matmul(
                            m1_ps[:, hh * C:(hh + 1) * C],
                            gt_sb[:, hh * CA:(hh + 1) * CA],
                            pvt,
                        )
                    m1_ps_p[ub] = m1_ps

                # --- stage C: l~ = I + (A m1)[:32,:], bias = (A m1)[32,:] ---
                ul = u - 2
                if 0 <= ul < NPAIR:
                    lb_ps = psl.tile([128, 8 * C + 2], F32, tag="l")
                    m1b = m1b_p[ul]
                    nc.tensor.matmul(lb_ps[0:C, 0:8 * C], at_sb[:, 0:C], m1b[:])
                    nc.tensor.matmul(lb_ps[:, 8 * C:8 * C + 1],
                                     m1b[:, 0:4 * C], at_sb[:, C:C + 1])
                    nc.tensor.matmul(lb_ps[:, 8 * C + 1:8 * C + 2],
                                     m1b[:, 4 * C:8 * C], at_sb[:, C:C + 1])
                    lb_ps_p[ul] = lb_ps

                # --- stage D: out4 = L4^T Q4 for the 2 blocks of pair u-4 ---
                uo = u - 4
                if 0 <= uo < NPAIR:
                    g, pu = uo // 2, uo % 2
                    qg = qgs[g]
                    o_psA = pso.tile([128, W], F32, tag="o")
                    nc.tensor.matmul(o_psA[:], l4ps[uo][:, 0:128],
                                     qg[:, (pu * 2) * W:(pu * 2 + 1) * W])
                    o_psB = pso.tile([128, W], F32, tag="o")
                    nc.tensor.matmul(o_psB[:], l4ps[uo][:, 128:256],
                                     qg[:, (pu * 2 + 1) * W:(pu * 2 + 2) * W])
                    o_psA_p[uo] = o_psA
                    o_psB_p[uo] = o_psB

                # --- copies, DVE in PE-completion order:
                #     gt(u), m1b(u-1), lbsb(u-2), scatter(u-2), bias(u-2)
                if u < NPAIR:
                    gt_sb = spool.tile([CA, 8 * CA], BF16, tag="gt")
                    nc.vector.tensor_copy(gt_sb[:], gt_ps_p[u][:])
                    gt_sb_p[u] = gt_sb
                if 0 <= ub < NPAIR:
                    m1b = spool.tile([CA, 8 * C], BF16, tag="m1b")
                    nc.vector.tensor_copy(m1b[:], m1_ps_p[ub][:])
                    m1b_p[ub] = m1b
                if 0 <= ul < NPAIR:
                    lb_ps = lb_ps_p[ul]
                    lbsb = spool.tile([C, 8 * C], BF16, tag="lbsb")
                    # l~ = (A m1) + I, identity folded into the PSUM->SBUF copy
                    nc.vector.tensor_add(lbsb[:], lb_ps[0:C, 0:8 * C], i32t8)
                    lbsb3 = lbsb.rearrange("p (b c) -> p b c", b=2)
                    l4p3 = l4ps[ul].rearrange("p (b c) -> p b c", b=2)
                    for i in range(4):
                        nc.gpsimd.tensor_copy(
                            l4p3[C * i:C * (i + 1), :, C * i:C * (i + 1)],
                            lbsb3[0:C, :, C * i:C * (i + 1)])
                    biasA = spool.tile([128, 1], F32, tag="biasA")
                    biasB = spool.tile([128, 1], F32, tag="biasB")
                    nc.vector.tensor_copy(biasA[:], lb_ps[:, 8 * C:8 * C + 1])
                    nc.vector.tensor_copy(biasB[:], lb_ps[:, 8 * C + 1:8 * C + 2])
                    biasA_p[ul] = biasA
                    biasB_p[ul] = biasB

                # --- output copies (+bias) on ACT, DMA on sync ---
                if 0 <= uo < NPAIR:
                    og = opool.tile([128, 2 * W], BF16, tag=f"og{uo}", name=f"og{uo}")
                    nc.vector.tensor_scalar_add(og[:, 0:W], o_psA_p[uo][:],
                                                biasA_p[uo][:])
                    nc.scalar.add(og[:, W:2 * W], o_psB_p[uo][:], biasB_p[uo][:])
                    nc.sync.dma_start(out_ap[:, uo * 2 * W:(uo + 1) * 2 * W], og[:])

    nc.compile()
    return nc


def _prep_core(qb, kb, vb):
    """Host-side relayout for one batch element (one core)."""
    # Q: 4-head channel stacking  [128, NBLK*W], partition = 32*(h%4) + c
    q4 = np.ascontiguousarray(
        qb.reshape(C, NBLK, 4, W).transpose(2, 0, 1, 3)
    ).reshape(128, NBLK * W).astype(NP_BF16)

    def tr(x):
        t = np.empty((HW, CA), dtype=np.float32)
        t[:, :C] = x.reshape(C, HW).T
        t[:, C] = 1.0
        return np.ascontiguousarray(
            t.reshape(NCHUNK, 128, CA).transpose(1, 0, 2)
        ).reshape(128, NCHUNK * CA).astype(NP_BF16)

    return q4, tr(kb), tr(vb)


def _install_ntff_hook():
    """Provide antenv.axon_hooks (absent in this image) so trace=True works."""
    import sys
    import types

    if "antenv.axon_hooks" in sys.modules:
        return
    try:
        import antenv
    except ImportError:
        return
    mod = types.ModuleType("antenv.axon_hooks")
    store = {}
    mod.set_axon_ntff_profile_hook = lambda h: store.__setitem__("h", h)
    mod.get_axon_ntff_profile_hook = lambda: store.get("h")
    sys.modules["antenv.axon_hooks"] = mod
    antenv.axon_hooks = mod
    try:
        from trn_agent_boot.trn_boot import _ntff_profile_via_ctypes

        hook = _ntff_profile_via_ctypes("/opt/axon/libaxon_pjrt.so")
        if hook is not None:
            store["h"] = hook
    except Exception:
        pass


def kernel(q, k, v, wq, bq, wk, bk, wv, bv):
    global last_exec_time_ns
    if "nc" not in _cache:
        _cache["nc"] = _build()
    nc = _cache["nc"]

    q = np.asarray(q, np.float32)
    k = np.asarray(k, np.float32)
    v = np.asarray(v, np.float32)
    wq = np.asarray(wq, np.float32)
    bq = np.asarray(bq, np.float32)
    wk = np.asarray(wk, np.float32)
    bk = np.asarray(bk, np.float32)
    wv = np.asarray(wv, np.float32)
    bv = np.asarray(bv, np.float32)

    cf = np.zeros((CA, 3 * CA), np.float32)
    cf[0:C, 0:CA] = np.concatenate([wq, bq[:, None]], axis=1)
    cf[0:C, CA:2 * CA] = np.concatenate([wk, bk[:, None]], axis=1)
    cf[:, 2 * CA:3 * CA] = np.eye(CA)
    cb = np.zeros((CA, 2 * C + 256), np.float32)
    cb[:, 0:C] = np.concatenate([wv.T, bv[None, :]], axis=0)
    cb[0:C, C:2 * C] = np.eye(C)
    cb[0:C, 2 * C:2 * C + 256] = np.tile(np.eye(C), (1, 8))
    cb = cb.astype(NP_BF16)

    in_maps = []
    for b in range(B):
        q4, kta, vta = _prep_core(q[b], k[b], v[b])
        in_maps.append({
            "q4": q4, "kta": kta, "vta": vta,
            "cf": cf, "cb": cb,
        })

    trace = os.environ.get("KERNEL_TRACE", "0") == "1"
    if trace:
        _install_ntff_hook()
    res = run_bass_kernel_spmd(nc, in_maps, core_ids=list(range(B)), trace=trace)
    last_exec_time_ns = res.exec_time_ns

    outs = []
    for b in range(B):
        arr = np.asarray(res.results[b]["out"], dtype=np.float32).reshape(4, C, H // 4, W)
        outs.append(np.transpose(arr, (1, 2, 0, 3)).reshape(C, H, W))
    return np.stack(outs).astype(np.float32)
